# revision 43
# baseline (speedup 1.0000x reference)
"""Trainium2 Bass kernel: Ernie4.5 VisionAttention (varlen attention, 4x512
segments, 16 heads x 80 dim, embed 1280).

Sharding: 8 cores = 2 segment-groups (2x512 tokens each) x 4 head-groups
(4 heads each). Tensor-parallel over heads (qkv column-shard, proj row-shard),
data-parallel over segment pairs. No collectives: per-core proj partials are
summed on the host.

Schedule: segment-software-pipelined single emission order so the PE never
idles across phase boundaries (qkv proj -> rotary -> attention -> out proj).
Softmax denominator is folded into V as a ones-column; per-head context is
packed to [320, 512] via SBUF DMA so the out-proj contracts full partitions.

Compute dtype: bf16 operands, fp32 PSUM accumulation (scores drain to bf16).
"""

import sys

if "/opt/trn_rl_repo" not in sys.path:
    sys.path.insert(0, "/opt/trn_rl_repo")

import numpy as np
import ml_dtypes

BF = ml_dtypes.bfloat16

EMBED = 1280
HEADS = 16
HD = 80          # head dim
RH = 40          # rotary half
SEQ = 2048
SEGLEN = 512
N_CORES = 8
HPC = 4          # heads per core
TOK = 1024       # tokens per core (2 segments)
NSEG = 2
NUNITS = 2 * HPC # q units 0..3, k units 4..7
VW = 97          # v block width per head: 80 v dims, pad, ones at col 96
ONESC = 96       # ones column -> softmax denom lands at partition 96 (32-mult)
VTOT = HPC * VW  # 388
CTXROWS = HPC * HD  # 320 packed context rows
SCALE = HD ** -0.5
KCH = EMBED // 128  # 10

_CACHE = {}


def _build_program(debug=False):
    import concourse.tile as tile
    from concourse import bacc, mybir

    f32 = mybir.dt.float32
    bf16 = mybir.dt.bfloat16
    AF = mybir.ActivationFunctionType
    ALU = mybir.AluOpType

    nc = bacc.Bacc("TRN2", target_bir_lowering=False, debug=False,
                   num_devices=N_CORES)

    # x.T and the t=0 wqk block side by side (first-group critical bytes);
    # wqk blocks t=1..4 arrive separately right behind
    xw_d = nc.dram_tensor("xw", [EMBED, TOK + 128], bf16,
                          kind="ExternalInput").ap()
    wqkh_d = nc.dram_tensor("wqkh", [EMBED, NUNITS * HD - 128], bf16,
                            kind="ExternalInput").ap()
    wv_d = nc.dram_tensor("wv", [EMBED, VTOT], bf16, kind="ExternalInput").ap()
    wppf_d = nc.dram_tensor("wppf", [128, 3 * EMBED], bf16, kind="ExternalInput").ap()
    bias_d = nc.dram_tensor("biasqk", [128, 5], f32, kind="ExternalInput").ap()
    cs_d = nc.dram_tensor("cs", [NUNITS * HD, 2 * TOK], bf16, kind="ExternalInput").ap()
    pit_d = nc.dram_tensor("pit", [NUNITS * HD, NUNITS * HD], bf16, kind="ExternalInput").ap()
    out_d = nc.dram_tensor("outT", [EMBED, TOK], bf16, kind="ExternalOutput").ap()
    if debug:
        dbg_qrot = nc.dram_tensor("dbg_qrot", [NUNITS * HD, TOK], bf16,
                                  kind="ExternalOutput").ap()
        dbg_v = nc.dram_tensor("dbg_v", [TOK, VTOT], bf16,
                               kind="ExternalOutput").ap()
        dbg_rec = nc.dram_tensor("dbg_rec", [NSEG * HPC, SEGLEN], f32,
                                 kind="ExternalOutput").ap()
        dbg_ctxp = nc.dram_tensor("dbg_ctxp", [NSEG * CTXROWS, SEGLEN], bf16,
                                  kind="ExternalOutput").ap()

    # packed-row unpack map: unit u rows [80u, 80u+80) live in 128-row tiles
    UNPACK_PIECES = {t: [] for t in range(5)}
    for u in range(NUNITS):
        a = HD * u
        while a < HD * (u + 1):
            t = a // 128
            b = min(HD * (u + 1), 128 * (t + 1))
            UNPACK_PIECES[t].append((u, a - HD * u, a - 128 * t, b - a))
            a = b
    # ctx pack map: head j rows [80j, 80j+80) -> packed tiles of 128
    PACK_PIECES = []  # (j, src_off, tile, dst_off, len)
    for j in range(HPC):
        a = HD * j
        while a < HD * (j + 1):
            t = a // 128
            b = min(HD * (j + 1), 128 * (t + 1))
            PACK_PIECES.append((j, a - HD * j, t, a - 128 * t, b - a))
            a = b
    # Pi-swap source blocks per packed target tile
    PI_BLOCKS = {0: [0, 1], 1: [0, 1, 2], 2: [1, 2, 3], 3: [2, 3, 4], 4: [3, 4]}
    NPT = CTXROWS // 128 + (1 if CTXROWS % 128 else 0)  # 3 packed ctx tiles
    PROWS = [min(128, CTXROWS - 128 * c) for c in range(NPT)]  # 128,128,64

    with tile.TileContext(nc) as tc:
        with tc.tile_pool(name="persist", bufs=1) as P, \
             tc.tile_pool(name="work", bufs=3) as W, \
             tc.tile_pool(name="ps_mm", bufs=2, space="PSUM") as PSM, \
             tc.tile_pool(name="ps_a", bufs=2, space="PSUM") as PSA, \
             tc.tile_pool(name="ps_e", bufs=2, space="PSUM") as PSE:

            # ---------------- persistent SBUF tiles ----------------
            bias_sb = P.tile([128, 5], f32, name="biasqk_sb", tag="biasqk")
            xw_sb = [P.tile([128, TOK + 128], bf16, name=f"xw{e}",
                            tag=f"xw{e}") for e in range(KCH)]
            xt_sb = [t[:, 0:TOK] for t in xw_sb]
            wqk0_sb = [t[:, TOK:TOK + 128] for t in xw_sb]
            wqkh_sb = [P.tile([128, NUNITS * HD - 128], bf16, name=f"wqkh{e}",
                              tag=f"wqkh{e}") for e in range(KCH)]
            warm_sb = P.tile([128, 128], bf16, name="warm_sb", tag="warm")
            ones80 = P.tile([1, HD], bf16, name="ones80", tag="ones80")
            wv_sb = [P.tile([128, VTOT], bf16, name=f"wv{e}", tag=f"wv{e}")
                     for e in range(KCH)]
            # cos|sin packed per 128-row tile: cols [cos 0:TOK | sin TOK:2TOK]
            cs_sb = [P.tile([128, 2 * TOK], bf16, name=f"cs{t}", tag=f"cs{t}")
                     for t in range(5)]

            def cosv(t, s):
                return cs_sb[t][:, SEGLEN * s:SEGLEN * (s + 1)]

            def sinv(t, s):
                return cs_sb[t][:, TOK + SEGLEN * s:TOK + SEGLEN * (s + 1)]

            pit_sb = [P.tile([128, NUNITS * HD], bf16, name=f"pit{t}", tag=f"pit{t}")
                      for t in range(5)]
            # wpp flattened: col block c holds packed ctx rows 128c..128c+PROWS[c]
            wppf_sb = P.tile([128, NPT * EMBED], bf16, name="wppf", tag="wppf")
            qrot = [P.tile([HD, TOK], bf16, name=f"qrot{u}", tag=f"qrot{u}")
                    for u in range(NUNITS)]
            v_sb = [P.tile([128, VTOT], bf16, name=f"vsb{m}", tag=f"vsb{m}")
                    for m in range(TOK // 128)]
            ctxp = [[P.tile([PROWS[c], SEGLEN], bf16, name=f"ctxp{c}_{s}",
                            tag=f"ctxp{c}_{s}") for c in range(NPT)]
                    for s in range(NSEG)]

            # ------------- input DMAs, ordered by need time ---------
            # Per-DMA queue occupancy is ~600ns regardless of size, so the
            # initial critical load (xt + wqk) fans out over the sync, scalar
            # and vector rings in parallel (all engines idle at t=0); later
            # inputs ride sync interleaved with the unpack/pack copies by
            # need time. Outputs drain via the gpsimd SWDGE ring.
            dma = nc.sync.dma_start

            def load_critical():
                rings = [nc.sync.dma_start, nc.scalar.dma_start,
                         nc.gpsimd.dma_start]
                q = []
                q.append((bias_sb[:], bias_d[:]))
                for e in range(KCH):
                    r = slice(128 * e, 128 * (e + 1))
                    q.append((xw_sb[e][:], xw_d[r, :]))
                for e in range(KCH):
                    r = slice(128 * e, 128 * (e + 1))
                    q.append((wqkh_sb[e][:], wqkh_d[r, :]))
                q.append((cs_sb[0][:], cs_d[0:128, :]))
                q.append((pit_sb[0][:], pit_d[0:128, :]))
                q.append((pit_sb[1][:], pit_d[128:256, :]))
                for i, (o, inp) in enumerate(q):
                    rings[i % 3](o, inp)

            def emit_warmup(n):
                # junk matmuls during the input load: keeps the PE HAM
                # activity window busy so real matmuls start at 2.4 GHz
                nc.vector.memset(warm_sb[:], 1.0)
                nc.vector.memset(ones80[:], 1.0)
                # pre-fill v tiles with 1.0: the per-head ones column (softmax
                # denominator) is preserved by the strided v copy in emit_v
                for m in range(TOK // 128):
                    nc.vector.memset(v_sb[m][:], 1.0)
                wps = PSM.tile([128, 128], f32, name="warmps", tag="mm512")
                for _ in range(n):
                    nc.tensor.matmul(wps[:], warm_sb[:], warm_sb[:],
                                     start=True, stop=True)

            def load_wv(es):
                for e in es:
                    dma(wv_sb[e][:], wv_d[128 * e:128 * (e + 1), :])

            def load_rot(t):
                dma(cs_sb[t][:], cs_d[128 * t:128 * (t + 1), :])

            def load_pit(t):
                dma(pit_sb[t][:], pit_d[128 * t:128 * (t + 1), :])

            def load_wpp():
                dma(wppf_sb[:], wppf_d[:])

            qkp_sb = [[None] * 5 for _ in range(NSEG)]

            # ------------------- emission helpers -------------------
            def emit_qk(t, s):
                sc = slice(SEGLEN * s, SEGLEN * (s + 1))
                qk_ps = PSM.tile([128, SEGLEN], f32, name=f"qkps{t}_{s}",
                                 tag="mm512")
                for e in range(KCH):
                    w = (wqk0_sb[e] if t == 0 else
                         wqkh_sb[e][:, 128 * (t - 1):128 * t])
                    nc.tensor.matmul(qk_ps[:], w, xt_sb[e][:, sc],
                                     start=(e == 0), stop=(e == KCH - 1))
                q_sb = W.tile([128, SEGLEN], bf16, name=f"qsb{t}_{s}",
                              tag="qsb", bufs=10)
                nc.scalar.activation(q_sb[:], qk_ps[:], AF.Identity,
                                     bias=bias_sb[:, t:t + 1])
                qkp_sb[s][t] = q_sb

            def emit_v(m):
                mc = slice(128 * m, 128 * (m + 1))
                v_ps = PSA.tile([128, SEGLEN], f32, name=f"vps{m}", tag="psa")
                for e in range(KCH):
                    nc.tensor.matmul(v_ps[:, 0:VTOT], xt_sb[e][:, mc],
                                     wv_sb[e][:], start=(e == 0),
                                     stop=(e == KCH - 1))
                nc.vector.tensor_copy(
                    v_sb[m][:].rearrange("p (h w) -> p h w", h=HPC)[:, :, 0:HD],
                    v_ps[:, 0:VTOT].rearrange("p (h w) -> p h w", h=HPC)[:, :, 0:HD])

            def emit_pi(tr, s):
                sc = slice(SEGLEN * s, SEGLEN * (s + 1))
                qsw_ps = PSA.tile([128, SEGLEN], f32, name=f"qsw{tr}_{s}",
                                  tag="psa")
                srcs = PI_BLOCKS[tr]
                for i, tp in enumerate(srcs):
                    nc.tensor.matmul(qsw_ps[:],
                                     pit_sb[tp][:, 128 * tr:128 * (tr + 1)],
                                     qkp_sb[s][tp][:],
                                     start=(i == 0), stop=(i == len(srcs) - 1))
                t2 = W.tile([128, SEGLEN], bf16, name=f"t2_{tr}_{s}", tag="t2",
                            bufs=3)
                nc.vector.tensor_tensor(t2[:], qsw_ps[:], sinv(tr, s),
                                        ALU.mult)
                t1 = W.tile([128, SEGLEN], bf16, name=f"t1_{tr}_{s}", tag="t1",
                            bufs=3)
                nc.vector.tensor_tensor(t1[:], qkp_sb[s][tr][:],
                                        cosv(tr, s), ALU.mult)
                rp = W.tile([128, SEGLEN], bf16, name=f"rotp{tr}_{s}",
                            tag="rotp", bufs=4)
                nc.vector.tensor_tensor(rp[:], t1[:], t2[:], ALU.add)
                for (u, po, toff, ln) in UNPACK_PIECES[tr]:
                    nc.sync.dma_start(qrot[u][po:po + ln, sc],
                                      rp[toff:toff + ln, :])

            def emit_attn(j, s):
                sc = slice(SEGLEN * s, SEGLEN * (s + 1))
                est = []
                for half in range(2):
                    e_ps = PSE.tile([128, 2 * SEGLEN], f32,
                                    name=f"eps{j}_{s}_{half}", tag="pse")
                    for q in range(2):
                        kc = slice(SEGLEN * s + 128 * (2 * half + q),
                                   SEGLEN * s + 128 * (2 * half + q) + 128)
                        nc.tensor.matmul(e_ps[:, SEGLEN * q:SEGLEN * (q + 1)],
                                         qrot[HPC + j][:, kc], qrot[j][:, sc],
                                         start=True, stop=True)
                    e_sb = W.tile([128, 2 * SEGLEN], bf16,
                                  name=f"est{j}_{s}_{half}", tag="est", bufs=4)
                    nc.scalar.activation(e_sb[:], e_ps[:], AF.Exp)
                    est.append(e_sb)
                ctx_ps = PSA.tile([128, SEGLEN], f32, name=f"ctxps{j}_{s}",
                                  tag="psa")
                for kc in range(4):
                    nc.tensor.matmul(ctx_ps[0:VW, :],
                                     v_sb[4 * s + kc][:, VW * j:VW * (j + 1)],
                                     est[kc // 2][:, SEGLEN * (kc % 2):
                                                   SEGLEN * (kc % 2 + 1)],
                                     start=(kc == 0), stop=(kc == 3))
                # the normalize tail (emit_norm) is deferred one schedule
                # block so its PE broadcast never stalls the PE queue
                den = W.tile([1, SEGLEN], f32, name=f"den{j}_{s}", tag="den",
                             bufs=2)
                nc.vector.tensor_copy(den[:], ctx_ps[ONESC:ONESC + 1, :])
                ctxc = W.tile([HD, SEGLEN], bf16, name=f"ctxc{j}_{s}",
                              tag="ctxc", bufs=3)
                nc.vector.tensor_copy(ctxc[:], ctx_ps[0:HD, :])
                rec = W.tile([1, SEGLEN], f32, name=f"rec{j}_{s}", tag="rec",
                             bufs=2)
                nc.vector.reciprocal_approx_fast(rec[:], den[:])
                recb = W.tile([1, SEGLEN], bf16, name=f"recb{j}_{s}",
                              tag="recb", bufs=2)
                nc.vector.tensor_copy(recb[:], rec[:])
                if debug:
                    nc.sync.dma_start(dbg_rec[s * HPC + j:s * HPC + j + 1, :],
                                      den[:])
                state = (ctxc, recb)
                return state

            def emit_norm(j, s, state):
                ctxc, recb = state
                bc_ps = PSM.tile([128, SEGLEN], f32, name=f"bcps{j}_{s}",
                                 tag="mm512")
                nc.tensor.matmul(bc_ps[0:HD, :], ones80[:], recb[:],
                                 start=True, stop=True)
                ctxn = W.tile([HD, SEGLEN], bf16, name=f"ctxn{j}_{s}",
                              tag="ctxn", bufs=3)
                nc.vector.tensor_tensor(ctxn[:], ctxc[:], bc_ps[0:HD, :],
                                        ALU.mult)
                for (jj, so, c, do, ln) in PACK_PIECES:
                    if jj == j:
                        nc.sync.dma_start(ctxp[s][c][do:do + ln, :],
                                          ctxn[so:so + ln, :])

            def emit_proj(e, s):
                o_ps = PSM.tile([128, SEGLEN], f32, name=f"ops{e}_{s}",
                                tag="mm512")
                for c in range(NPT):
                    nc.tensor.matmul(o_ps[:],
                                     wppf_sb[0:PROWS[c],
                                             EMBED * c + 128 * e:
                                             EMBED * c + 128 * (e + 1)],
                                     ctxp[s][c][:],
                                     start=(c == 0), stop=(c == NPT - 1))
                o_sb = W.tile([128, SEGLEN], bf16, name=f"osb{e}_{s}",
                              tag="osb", bufs=4)
                if (e + s) % 2 == 0:
                    nc.vector.tensor_copy(o_sb[:], o_ps[:])
                else:
                    nc.scalar.copy(o_sb[:], o_ps[:])
                oring = nc.sync.dma_start if e % 2 == 0 else nc.gpsimd.dma_start
                oring(
                    out_d[128 * e:128 * (e + 1), SEGLEN * s:SEGLEN * (s + 1)],
                    o_sb[:])

            # -------------------- global schedule -------------------
            load_critical()
            emit_warmup(34)
            def keep_warm(n):
                wps = PSM.tile([128, 128], f32, name="warmps2", tag="mm512")
                for _ in range(n):
                    nc.tensor.matmul(wps[:], warm_sb[:], warm_sb[:],
                                     start=True, stop=True)

            emit_qk(0, 0); keep_warm(6)
            emit_qk(1, 0); load_rot(1); load_pit(2); keep_warm(6)
            emit_qk(2, 0); load_rot(2); load_pit(3)
            emit_pi(0, 0); keep_warm(4)
            emit_qk(3, 0); load_wv(range(0, 5))
            emit_pi(1, 0); load_rot(3); load_pit(4)
            emit_qk(4, 0); load_wv(range(5, 10))
            emit_pi(2, 0); load_rot(4)
            emit_pi(3, 0)
            emit_pi(4, 0)
            emit_v(0); emit_v(1)
            emit_v(2); emit_v(3)

            emit_qk(0, 1)
            emit_qk(1, 1)
            emit_pi(0, 1)
            emit_qk(2, 1); emit_pi(1, 1)

            st = emit_attn(0, 0); load_wpp()
            emit_qk(3, 1); emit_pi(2, 1)
            emit_norm(0, 0, st)
            st = emit_attn(1, 0)
            emit_qk(4, 1); emit_pi(3, 1); emit_pi(4, 1)
            emit_norm(1, 0, st)
            st = emit_attn(2, 0)
            emit_v(4); emit_v(5)
            emit_norm(2, 0, st)
            st = emit_attn(3, 0)
            emit_v(6); emit_v(7)
            emit_norm(3, 0, st)

            st = emit_attn(0, 1)
            emit_proj(0, 0)
            emit_norm(0, 1, st)
            emit_proj(1, 0)
            st = emit_attn(1, 1)
            emit_proj(2, 0)
            emit_norm(1, 1, st)
            emit_proj(3, 0)
            st = emit_attn(2, 1)
            emit_proj(4, 0)
            emit_norm(2, 1, st)
            emit_proj(5, 0)
            st = emit_attn(3, 1)
            emit_proj(6, 0); emit_proj(7, 0)
            emit_norm(3, 1, st)
            emit_proj(8, 0); emit_proj(9, 0)
            for e in range(KCH):
                emit_proj(e, 1)

            if debug:
                for u in range(NUNITS):
                    nc.sync.dma_start(dbg_qrot[HD * u:HD * (u + 1), :],
                                      qrot[u][:])
                for m in range(TOK // 128):
                    nc.sync.dma_start(dbg_v[128 * m:128 * (m + 1), :],
                                      v_sb[m][:])
                for s in range(NSEG):
                    ro = 0
                    for c in range(NPT):
                        nc.sync.dma_start(
                            dbg_ctxp[CTXROWS * s + ro:CTXROWS * s + ro + PROWS[c], :],
                            ctxp[s][c][:])
                        ro += PROWS[c]

    nc.compile()
    return nc


def _prep_inputs(x, rotary_pos_emb, qkv_w, qkv_b):
    """Build per-core input shards (host-side layout/constant prep)."""
    x2 = np.asarray(x, np.float32).reshape(SEQ, EMBED)
    rope = np.asarray(rotary_pos_emb, np.float32)
    qkv_w = np.asarray(qkv_w, np.float32)
    qkv_b = np.asarray(qkv_b, np.float32)

    # packed rotary multipliers: packed row p = 80u + d -> r = d % 40
    r_idx = np.tile(np.arange(HD) % RH, NUNITS)      # [640]
    cos_full = np.cos(rope)[:, r_idx].T.astype(BF)   # [640, 2048]
    sin_full = np.sin(rope)[:, r_idx].T.astype(BF)

    # packed swap permutation (sign folded), block-diagonal per 80-row unit:
    # within a unit, row d<40 reads -(d+40), row d>=40 reads +(d-40)
    D = NUNITS * HD
    Pi = np.zeros((D, D), np.float32)
    for u in range(NUNITS):
        o = HD * u
        for i in range(RH):
            Pi[o + i, o + i + RH] = -1.0
            Pi[o + i + RH, o + i] = 1.0
    pit = np.ascontiguousarray(Pi.T).astype(BF)

    in_maps = []
    for c in range(N_CORES):
        sg, hg = divmod(c, HPC)
        toks = slice(TOK * sg, TOK * (sg + 1))
        heads = [HPC * hg + j for j in range(HPC)]

        xw = np.empty((EMBED, TOK + 128), np.float32)
        xw[:, 0:TOK] = x2[toks].T
        wqk = np.empty((EMBED, NUNITS * HD), np.float32)
        bias_flat = np.empty(NUNITS * HD, np.float32)
        for j, h in enumerate(heads):
            uq, uk = j, HPC + j
            wqk[:, HD * uq:HD * (uq + 1)] = qkv_w[HD * h:HD * (h + 1), :].T * SCALE
            bias_flat[HD * uq:HD * (uq + 1)] = qkv_b[HD * h:HD * (h + 1)] * SCALE
            ko = EMBED + HD * h
            wqk[:, HD * uk:HD * (uk + 1)] = qkv_w[ko:ko + HD, :].T
            bias_flat[HD * uk:HD * (uk + 1)] = qkv_b[ko:ko + HD]
        bias = np.ascontiguousarray(bias_flat.reshape(5, 128).T)

        # v weights (v bias is zero per setup_inputs; ones column comes
        # from the device-side memset of v_sb)
        assert not np.any(qkv_b[2 * EMBED:]), "nonzero v bias unsupported"
        wv = np.zeros((EMBED, VTOT), np.float32)
        for j, h in enumerate(heads):
            vo = 2 * EMBED + HD * h
            wv[:, VW * j:VW * j + HD] = qkv_w[vo:vo + HD, :].T

        # proj weights packed by head then flattened into one 128-row tile:
        # col block c = packed ctx rows 128c..128c+PROWS_c
        wpp = np.empty((CTXROWS, EMBED), np.float32)
        for j, h in enumerate(heads):
            wpp[HD * j:HD * (j + 1), :] = _PROJ_W[:, HD * h:HD * (h + 1)].T
        wppf = np.zeros((128, 3 * EMBED), np.float32)
        for cc in range(3):
            pr = min(128, CTXROWS - 128 * cc)
            wppf[0:pr, EMBED * cc:EMBED * (cc + 1)] = wpp[128 * cc:128 * cc + pr]

        cs = np.concatenate([cos_full[:, toks], sin_full[:, toks]],
                            axis=1).astype(BF)

        xw[:, TOK:TOK + 128] = wqk[:, 0:128]
        in_maps.append({
            "xw": xw.astype(BF),
            "wqkh": np.ascontiguousarray(wqk[:, 128:]).astype(BF),
            "wv": wv.astype(BF),
            "wppf": wppf.astype(BF),
            "biasqk": bias,
            "cs": np.ascontiguousarray(cs),
            "pit": pit,
        })
    return in_maps


_PROJ_W = None


def run_on_device(inputs, trace=False, trace_cores=None):
    """Shard, run on 8 NeuronCores, gather. Returns (output, BassKernelResults)."""
    global _PROJ_W
    from concourse import bass_utils

    x = np.asarray(inputs["x"], np.float32)
    cu = np.asarray(inputs["cu_seqlens"]).tolist()
    assert cu == [0, 512, 1024, 1536, 2048], (
        f"kernel compiled for 4x512 segments, got cu_seqlens={cu}")
    assert x.shape == (SEQ, 1, EMBED)

    _PROJ_W = np.asarray(inputs["proj_w"], np.float32)
    in_maps = _prep_inputs(x, inputs["rotary_pos_emb"],
                           inputs["qkv_w"], inputs["qkv_b"])

    if "nc" not in _CACHE:
        _CACHE["nc"] = _build_program()
    nc = _CACHE["nc"]

    kw = {}
    if trace:
        kw = dict(trace=True, trace_cores=trace_cores or [0])
    res = bass_utils.run_bass_kernel_spmd(nc, in_maps,
                                          core_ids=list(range(N_CORES)), **kw)

    proj_b = np.asarray(inputs["proj_b"], np.float32)
    out = np.empty((SEQ, EMBED), np.float32)
    for sg in range(2):
        acc = res.results[HPC * sg + 0]["outT"].astype(np.float32).copy()
        for hg in range(1, HPC):
            acc += res.results[HPC * sg + hg]["outT"].astype(np.float32)
        out[TOK * sg:TOK * (sg + 1)] = acc.T
    out += proj_b
    return out.reshape(SEQ, 1, EMBED), res


def kernel(**inputs):
    out, _ = run_on_device(inputs, trace=False)
    return out


# revision 44
# speedup vs baseline: 1.0535x; 1.0535x over previous
"""Trainium2 Bass kernel: Ernie4.5 VisionAttention (varlen attention, 4x512
segments, 16 heads x 80 dim, embed 1280).

Sharding: 8 cores = 2 segment-groups (2x512 tokens each) x 4 head-groups
(4 heads each). Tensor-parallel over heads (qkv column-shard, proj row-shard),
data-parallel over segment pairs. No collectives: per-core proj partials are
summed on the host.

Schedule: segment-software-pipelined single emission order so the PE never
idles across phase boundaries (qkv proj -> rotary -> attention -> out proj).
Softmax denominator is folded into V as a ones-column; per-head context is
packed to [320, 512] via SBUF DMA so the out-proj contracts full partitions.

Compute dtype: bf16 operands, fp32 PSUM accumulation (scores drain to bf16).
"""

import sys

if "/opt/trn_rl_repo" not in sys.path:
    sys.path.insert(0, "/opt/trn_rl_repo")

import numpy as np
import ml_dtypes

BF = ml_dtypes.bfloat16

EMBED = 1280
HEADS = 16
HD = 80          # head dim
RH = 40          # rotary half
SEQ = 2048
SEGLEN = 512
N_CORES = 8
HPC = 4          # heads per core
TOK = 1024       # tokens per core (2 segments)
NSEG = 2
NUNITS = 2 * HPC # q units 0..3, k units 4..7
VW = 97          # v block width per head: 80 v dims, pad, ones at col 96
ONESC = 96       # ones column -> softmax denom lands at partition 96 (32-mult)
VTOT = HPC * VW  # 388
CTXROWS = HPC * HD  # 320 packed context rows
SCALE = HD ** -0.5
KCH = EMBED // 128  # 10

_CACHE = {}


def _build_program(debug=False):
    import concourse.tile as tile
    from concourse import bacc, mybir

    f32 = mybir.dt.float32
    bf16 = mybir.dt.bfloat16
    AF = mybir.ActivationFunctionType
    ALU = mybir.AluOpType

    nc = bacc.Bacc("TRN2", target_bir_lowering=False, debug=False,
                   num_devices=N_CORES)

    # x.T and the t=0 wqk block side by side (first-group critical bytes);
    # wqk blocks t=1..4 arrive separately right behind
    xw_d = nc.dram_tensor("xw", [EMBED, TOK + 128], bf16,
                          kind="ExternalInput").ap()
    wqkh_d = nc.dram_tensor("wqkh", [EMBED, NUNITS * HD - 128], bf16,
                            kind="ExternalInput").ap()
    wv_d = nc.dram_tensor("wv", [EMBED, VTOT], bf16, kind="ExternalInput").ap()
    wppf_d = nc.dram_tensor("wppf", [128, 3 * EMBED], bf16, kind="ExternalInput").ap()
    bias_d = nc.dram_tensor("biasqk", [128, 5], f32, kind="ExternalInput").ap()
    cs_d = nc.dram_tensor("cs", [NUNITS * HD, 2 * TOK], bf16, kind="ExternalInput").ap()
    pit_d = nc.dram_tensor("pit", [NUNITS * HD, NUNITS * HD], bf16, kind="ExternalInput").ap()
    out_d = nc.dram_tensor("outT", [EMBED, TOK], bf16, kind="ExternalOutput").ap()
    if debug:
        dbg_qrot = nc.dram_tensor("dbg_qrot", [NUNITS * HD, TOK], bf16,
                                  kind="ExternalOutput").ap()
        dbg_v = nc.dram_tensor("dbg_v", [TOK, VTOT], bf16,
                               kind="ExternalOutput").ap()
        dbg_rec = nc.dram_tensor("dbg_rec", [NSEG * HPC, SEGLEN], f32,
                                 kind="ExternalOutput").ap()
        dbg_ctxp = nc.dram_tensor("dbg_ctxp", [NSEG * CTXROWS, SEGLEN], bf16,
                                  kind="ExternalOutput").ap()

    # packed-row unpack map: unit u rows [80u, 80u+80) live in 128-row tiles
    UNPACK_PIECES = {t: [] for t in range(5)}
    for u in range(NUNITS):
        a = HD * u
        while a < HD * (u + 1):
            t = a // 128
            b = min(HD * (u + 1), 128 * (t + 1))
            UNPACK_PIECES[t].append((u, a - HD * u, a - 128 * t, b - a))
            a = b
    # ctx pack map: head j rows [80j, 80j+80) -> packed tiles of 128
    PACK_PIECES = []  # (j, src_off, tile, dst_off, len)
    for j in range(HPC):
        a = HD * j
        while a < HD * (j + 1):
            t = a // 128
            b = min(HD * (j + 1), 128 * (t + 1))
            PACK_PIECES.append((j, a - HD * j, t, a - 128 * t, b - a))
            a = b
    # Pi-swap source blocks per packed target tile
    PI_BLOCKS = {0: [0, 1], 1: [0, 1, 2], 2: [1, 2, 3], 3: [2, 3, 4], 4: [3, 4]}
    NPT = CTXROWS // 128 + (1 if CTXROWS % 128 else 0)  # 3 packed ctx tiles
    PROWS = [min(128, CTXROWS - 128 * c) for c in range(NPT)]  # 128,128,64

    with tile.TileContext(nc) as tc:
        with tc.tile_pool(name="persist", bufs=1) as P, \
             tc.tile_pool(name="work", bufs=3) as W, \
             tc.tile_pool(name="ps_mm", bufs=2, space="PSUM") as PSM, \
             tc.tile_pool(name="ps_a", bufs=2, space="PSUM") as PSA, \
             tc.tile_pool(name="ps_e", bufs=2, space="PSUM") as PSE:

            # ---------------- persistent SBUF tiles ----------------
            bias_sb = P.tile([128, 5], f32, name="biasqk_sb", tag="biasqk")
            xw_sb = [P.tile([128, TOK + 128], bf16, name=f"xw{e}",
                            tag=f"xw{e}") for e in range(KCH)]
            xt_sb = [t[:, 0:TOK] for t in xw_sb]
            wqk0_sb = [t[:, TOK:TOK + 128] for t in xw_sb]
            wqkh_sb = [P.tile([128, NUNITS * HD - 128], bf16, name=f"wqkh{e}",
                              tag=f"wqkh{e}") for e in range(KCH)]
            warm_sb = P.tile([128, 128], bf16, name="warm_sb", tag="warm")
            ones80 = P.tile([1, HD], bf16, name="ones80", tag="ones80")
            wv_sb = [P.tile([128, VTOT], bf16, name=f"wv{e}", tag=f"wv{e}")
                     for e in range(KCH)]
            # cos|sin packed per 128-row tile: cols [cos 0:TOK | sin TOK:2TOK]
            cs_sb = [P.tile([128, 2 * TOK], bf16, name=f"cs{t}", tag=f"cs{t}")
                     for t in range(5)]

            def cosv(t, s):
                return cs_sb[t][:, SEGLEN * s:SEGLEN * (s + 1)]

            def sinv(t, s):
                return cs_sb[t][:, TOK + SEGLEN * s:TOK + SEGLEN * (s + 1)]

            pit_sb = [P.tile([128, NUNITS * HD], bf16, name=f"pit{t}", tag=f"pit{t}")
                      for t in range(5)]
            # wpp flattened: col block c holds packed ctx rows 128c..128c+PROWS[c]
            wppf_sb = P.tile([128, NPT * EMBED], bf16, name="wppf", tag="wppf")
            qrot = [P.tile([HD, TOK], bf16, name=f"qrot{u}", tag=f"qrot{u}")
                    for u in range(NUNITS)]
            v_sb = [P.tile([128, VTOT], bf16, name=f"vsb{m}", tag=f"vsb{m}")
                    for m in range(TOK // 128)]
            ctxp = [[P.tile([PROWS[c], SEGLEN], bf16, name=f"ctxp{c}_{s}",
                            tag=f"ctxp{c}_{s}") for c in range(NPT)]
                    for s in range(NSEG)]

            # ------------- input DMAs, ordered by need time ---------
            # Per-DMA queue occupancy is ~600ns regardless of size, so the
            # initial critical load (xt + wqk) fans out over the sync, scalar
            # and vector rings in parallel (all engines idle at t=0); later
            # inputs ride sync interleaved with the unpack/pack copies by
            # need time. Outputs drain via the gpsimd SWDGE ring.
            dma = nc.sync.dma_start

            def load_critical():
                rings = [nc.sync.dma_start, nc.scalar.dma_start,
                         nc.gpsimd.dma_start]
                q = []
                q.append((bias_sb[:], bias_d[:]))
                for e in range(KCH):
                    r = slice(128 * e, 128 * (e + 1))
                    q.append((xw_sb[e][:], xw_d[r, :]))
                for e in range(KCH):
                    r = slice(128 * e, 128 * (e + 1))
                    q.append((wqkh_sb[e][:], wqkh_d[r, :]))
                q.append((cs_sb[0][:], cs_d[0:128, :]))
                q.append((pit_sb[0][:], pit_d[0:128, :]))
                q.append((pit_sb[1][:], pit_d[128:256, :]))
                for i, (o, inp) in enumerate(q):
                    rings[i % 3](o, inp)

            def emit_warmup(n):
                # junk matmuls during the input load: keeps the PE HAM
                # activity window busy so real matmuls start at 2.4 GHz
                nc.vector.memset(warm_sb[:], 1.0)
                nc.vector.memset(ones80[:], 1.0)
                # pre-fill v tiles with 1.0: the per-head ones column (softmax
                # denominator) is preserved by the strided v copy in emit_v
                for m in range(TOK // 128):
                    nc.vector.memset(v_sb[m][:], 1.0)
                wps = PSM.tile([128, 128], f32, name="warmps", tag="mm512")
                for _ in range(n):
                    nc.tensor.matmul(wps[:], warm_sb[:], warm_sb[:],
                                     start=True, stop=True)

            def load_wv(es):
                for e in es:
                    dma(wv_sb[e][:], wv_d[128 * e:128 * (e + 1), :])

            def load_rot(t):
                dma(cs_sb[t][:], cs_d[128 * t:128 * (t + 1), :])

            def load_pit(t):
                dma(pit_sb[t][:], pit_d[128 * t:128 * (t + 1), :])

            def load_wpp():
                dma(wppf_sb[:], wppf_d[:])

            qkp_sb = [[None] * 5 for _ in range(NSEG)]

            # ------------------- emission helpers -------------------
            def emit_qk(t, s):
                sc = slice(SEGLEN * s, SEGLEN * (s + 1))
                qk_ps = PSM.tile([128, SEGLEN], f32, name=f"qkps{t}_{s}",
                                 tag="mm512")
                for e in range(KCH):
                    w = (wqk0_sb[e] if t == 0 else
                         wqkh_sb[e][:, 128 * (t - 1):128 * t])
                    nc.tensor.matmul(qk_ps[:], w, xt_sb[e][:, sc],
                                     start=(e == 0), stop=(e == KCH - 1))
                q_sb = W.tile([128, SEGLEN], bf16, name=f"qsb{t}_{s}",
                              tag="qsb", bufs=10)
                nc.scalar.activation(q_sb[:], qk_ps[:], AF.Identity,
                                     bias=bias_sb[:, t:t + 1])
                qkp_sb[s][t] = q_sb

            def emit_v(m):
                mc = slice(128 * m, 128 * (m + 1))
                v_ps = PSA.tile([128, SEGLEN], f32, name=f"vps{m}", tag="psa")
                for e in range(KCH):
                    nc.tensor.matmul(v_ps[:, 0:VTOT], xt_sb[e][:, mc],
                                     wv_sb[e][:], start=(e == 0),
                                     stop=(e == KCH - 1))
                nc.vector.tensor_copy(
                    v_sb[m][:].rearrange("p (h w) -> p h w", h=HPC)[:, :, 0:HD],
                    v_ps[:, 0:VTOT].rearrange("p (h w) -> p h w", h=HPC)[:, :, 0:HD])

            def emit_pi(tr, s):
                sc = slice(SEGLEN * s, SEGLEN * (s + 1))
                qsw_ps = PSA.tile([128, SEGLEN], f32, name=f"qsw{tr}_{s}",
                                  tag="psa")
                srcs = PI_BLOCKS[tr]
                for i, tp in enumerate(srcs):
                    nc.tensor.matmul(qsw_ps[:],
                                     pit_sb[tp][:, 128 * tr:128 * (tr + 1)],
                                     qkp_sb[s][tp][:],
                                     start=(i == 0), stop=(i == len(srcs) - 1))
                t2 = W.tile([128, SEGLEN], bf16, name=f"t2_{tr}_{s}", tag="t2",
                            bufs=3)
                nc.vector.tensor_tensor(t2[:], qsw_ps[:], sinv(tr, s),
                                        ALU.mult)
                t1 = W.tile([128, SEGLEN], bf16, name=f"t1_{tr}_{s}", tag="t1",
                            bufs=3)
                nc.vector.tensor_tensor(t1[:], qkp_sb[s][tr][:],
                                        cosv(tr, s), ALU.mult)
                rp = W.tile([128, SEGLEN], bf16, name=f"rotp{tr}_{s}",
                            tag="rotp", bufs=4)
                nc.vector.tensor_tensor(rp[:], t1[:], t2[:], ALU.add)
                for (u, po, toff, ln) in UNPACK_PIECES[tr]:
                    nc.sync.dma_start(qrot[u][po:po + ln, sc],
                                      rp[toff:toff + ln, :])

            def emit_attn(j, s):
                sc = slice(SEGLEN * s, SEGLEN * (s + 1))
                est = []
                for half in range(2):
                    e_ps = PSE.tile([128, 2 * SEGLEN], f32,
                                    name=f"eps{j}_{s}_{half}", tag="pse")
                    for q in range(2):
                        kc = slice(SEGLEN * s + 128 * (2 * half + q),
                                   SEGLEN * s + 128 * (2 * half + q) + 128)
                        nc.tensor.matmul(e_ps[:, SEGLEN * q:SEGLEN * (q + 1)],
                                         qrot[HPC + j][:, kc], qrot[j][:, sc],
                                         start=True, stop=True)
                    e_sb = W.tile([128, 2 * SEGLEN], bf16,
                                  name=f"est{j}_{s}_{half}", tag="est", bufs=4)
                    nc.scalar.activation(e_sb[:], e_ps[:], AF.Exp)
                    est.append(e_sb)
                ctx_ps = PSA.tile([128, SEGLEN], f32, name=f"ctxps{j}_{s}",
                                  tag="psa")
                for kc in range(4):
                    nc.tensor.matmul(ctx_ps[0:VW, :],
                                     v_sb[4 * s + kc][:, VW * j:VW * (j + 1)],
                                     est[kc // 2][:, SEGLEN * (kc % 2):
                                                   SEGLEN * (kc % 2 + 1)],
                                     start=(kc == 0), stop=(kc == 3))
                # the normalize tail (emit_norm) is deferred one schedule
                # block so its PE broadcast never stalls the PE queue
                den = W.tile([1, SEGLEN], f32, name=f"den{j}_{s}", tag="den",
                             bufs=2)
                nc.vector.tensor_copy(den[:], ctx_ps[ONESC:ONESC + 1, :])
                ctxc = W.tile([HD, SEGLEN], bf16, name=f"ctxc{j}_{s}",
                              tag="ctxc", bufs=3)
                nc.vector.tensor_copy(ctxc[:], ctx_ps[0:HD, :])
                rec = W.tile([1, SEGLEN], f32, name=f"rec{j}_{s}", tag="rec",
                             bufs=2)
                nc.vector.reciprocal_approx_fast(rec[:], den[:])
                recb = W.tile([1, SEGLEN], bf16, name=f"recb{j}_{s}",
                              tag="recb", bufs=2)
                nc.vector.tensor_copy(recb[:], rec[:])
                if debug:
                    nc.sync.dma_start(dbg_rec[s * HPC + j:s * HPC + j + 1, :],
                                      den[:])
                state = (ctxc, recb)
                return state

            def emit_norm(j, s, state):
                ctxc, recb = state
                bc_ps = PSM.tile([128, SEGLEN], f32, name=f"bcps{j}_{s}",
                                 tag="mm512")
                nc.tensor.matmul(bc_ps[0:HD, :], ones80[:], recb[:],
                                 start=True, stop=True)
                ctxn = W.tile([HD, SEGLEN], bf16, name=f"ctxn{j}_{s}",
                              tag="ctxn", bufs=3)
                nc.vector.tensor_tensor(ctxn[:], ctxc[:], bc_ps[0:HD, :],
                                        ALU.mult)
                for (jj, so, c, do, ln) in PACK_PIECES:
                    if jj == j:
                        nc.sync.dma_start(ctxp[s][c][do:do + ln, :],
                                          ctxn[so:so + ln, :])

            def emit_proj(e, s):
                o_ps = PSM.tile([128, SEGLEN], f32, name=f"ops{e}_{s}",
                                tag="mm512")
                for c in range(NPT):
                    nc.tensor.matmul(o_ps[:],
                                     wppf_sb[0:PROWS[c],
                                             EMBED * c + 128 * e:
                                             EMBED * c + 128 * (e + 1)],
                                     ctxp[s][c][:],
                                     start=(c == 0), stop=(c == NPT - 1))
                o_sb = W.tile([128, SEGLEN], bf16, name=f"osb{e}_{s}",
                              tag="osb", bufs=4)
                if (e + s) % 2 == 0:
                    nc.vector.tensor_copy(o_sb[:], o_ps[:])
                else:
                    nc.scalar.copy(o_sb[:], o_ps[:])
                oring = nc.sync.dma_start if e % 2 == 0 else nc.gpsimd.dma_start
                oring(
                    out_d[128 * e:128 * (e + 1), SEGLEN * s:SEGLEN * (s + 1)],
                    o_sb[:])

            # -------------------- global schedule -------------------
            load_critical()
            emit_warmup(24)
            emit_qk(0, 0)
            emit_qk(1, 0); load_rot(1); load_pit(2)
            emit_qk(2, 0); load_rot(2); load_pit(3)
            emit_pi(0, 0)
            emit_qk(3, 0); load_wv(range(0, 5))
            emit_pi(1, 0); load_rot(3); load_pit(4)
            emit_qk(4, 0); load_wv(range(5, 10))
            emit_pi(2, 0); load_rot(4)
            emit_pi(3, 0)
            emit_pi(4, 0)
            emit_v(0); emit_v(1)
            emit_v(2); emit_v(3)

            emit_qk(0, 1)
            emit_qk(1, 1)
            emit_pi(0, 1)
            emit_qk(2, 1); emit_pi(1, 1)

            st = emit_attn(0, 0); load_wpp()
            emit_qk(3, 1); emit_pi(2, 1)
            emit_norm(0, 0, st)
            st = emit_attn(1, 0)
            emit_qk(4, 1); emit_pi(3, 1); emit_pi(4, 1)
            emit_norm(1, 0, st)
            st = emit_attn(2, 0)
            emit_v(4); emit_v(5)
            emit_norm(2, 0, st)
            st = emit_attn(3, 0)
            emit_v(6); emit_v(7)
            emit_norm(3, 0, st)

            st = emit_attn(0, 1)
            emit_proj(0, 0)
            emit_norm(0, 1, st)
            emit_proj(1, 0)
            st = emit_attn(1, 1)
            emit_proj(2, 0)
            emit_norm(1, 1, st)
            emit_proj(3, 0)
            st = emit_attn(2, 1)
            emit_proj(4, 0)
            emit_norm(2, 1, st)
            emit_proj(5, 0)
            st = emit_attn(3, 1)
            emit_proj(6, 0); emit_proj(7, 0)
            emit_norm(3, 1, st)
            emit_proj(8, 0); emit_proj(9, 0)
            for e in range(KCH):
                emit_proj(e, 1)

            if debug:
                for u in range(NUNITS):
                    nc.sync.dma_start(dbg_qrot[HD * u:HD * (u + 1), :],
                                      qrot[u][:])
                for m in range(TOK // 128):
                    nc.sync.dma_start(dbg_v[128 * m:128 * (m + 1), :],
                                      v_sb[m][:])
                for s in range(NSEG):
                    ro = 0
                    for c in range(NPT):
                        nc.sync.dma_start(
                            dbg_ctxp[CTXROWS * s + ro:CTXROWS * s + ro + PROWS[c], :],
                            ctxp[s][c][:])
                        ro += PROWS[c]

    nc.compile()
    return nc


def _prep_inputs(x, rotary_pos_emb, qkv_w, qkv_b):
    """Build per-core input shards (host-side layout/constant prep)."""
    x2 = np.asarray(x, np.float32).reshape(SEQ, EMBED)
    rope = np.asarray(rotary_pos_emb, np.float32)
    qkv_w = np.asarray(qkv_w, np.float32)
    qkv_b = np.asarray(qkv_b, np.float32)

    # packed rotary multipliers: packed row p = 80u + d -> r = d % 40
    r_idx = np.tile(np.arange(HD) % RH, NUNITS)      # [640]
    cos_full = np.cos(rope)[:, r_idx].T.astype(BF)   # [640, 2048]
    sin_full = np.sin(rope)[:, r_idx].T.astype(BF)

    # packed swap permutation (sign folded), block-diagonal per 80-row unit:
    # within a unit, row d<40 reads -(d+40), row d>=40 reads +(d-40)
    D = NUNITS * HD
    Pi = np.zeros((D, D), np.float32)
    for u in range(NUNITS):
        o = HD * u
        for i in range(RH):
            Pi[o + i, o + i + RH] = -1.0
            Pi[o + i + RH, o + i] = 1.0
    pit = np.ascontiguousarray(Pi.T).astype(BF)

    in_maps = []
    for c in range(N_CORES):
        sg, hg = divmod(c, HPC)
        toks = slice(TOK * sg, TOK * (sg + 1))
        heads = [HPC * hg + j for j in range(HPC)]

        xw = np.empty((EMBED, TOK + 128), np.float32)
        xw[:, 0:TOK] = x2[toks].T
        wqk = np.empty((EMBED, NUNITS * HD), np.float32)
        bias_flat = np.empty(NUNITS * HD, np.float32)
        for j, h in enumerate(heads):
            uq, uk = j, HPC + j
            wqk[:, HD * uq:HD * (uq + 1)] = qkv_w[HD * h:HD * (h + 1), :].T * SCALE
            bias_flat[HD * uq:HD * (uq + 1)] = qkv_b[HD * h:HD * (h + 1)] * SCALE
            ko = EMBED + HD * h
            wqk[:, HD * uk:HD * (uk + 1)] = qkv_w[ko:ko + HD, :].T
            bias_flat[HD * uk:HD * (uk + 1)] = qkv_b[ko:ko + HD]
        bias = np.ascontiguousarray(bias_flat.reshape(5, 128).T)

        # v weights (v bias is zero per setup_inputs; ones column comes
        # from the device-side memset of v_sb)
        assert not np.any(qkv_b[2 * EMBED:]), "nonzero v bias unsupported"
        wv = np.zeros((EMBED, VTOT), np.float32)
        for j, h in enumerate(heads):
            vo = 2 * EMBED + HD * h
            wv[:, VW * j:VW * j + HD] = qkv_w[vo:vo + HD, :].T

        # proj weights packed by head then flattened into one 128-row tile:
        # col block c = packed ctx rows 128c..128c+PROWS_c
        wpp = np.empty((CTXROWS, EMBED), np.float32)
        for j, h in enumerate(heads):
            wpp[HD * j:HD * (j + 1), :] = _PROJ_W[:, HD * h:HD * (h + 1)].T
        wppf = np.zeros((128, 3 * EMBED), np.float32)
        for cc in range(3):
            pr = min(128, CTXROWS - 128 * cc)
            wppf[0:pr, EMBED * cc:EMBED * (cc + 1)] = wpp[128 * cc:128 * cc + pr]

        cs = np.concatenate([cos_full[:, toks], sin_full[:, toks]],
                            axis=1).astype(BF)

        xw[:, TOK:TOK + 128] = wqk[:, 0:128]
        in_maps.append({
            "xw": xw.astype(BF),
            "wqkh": np.ascontiguousarray(wqk[:, 128:]).astype(BF),
            "wv": wv.astype(BF),
            "wppf": wppf.astype(BF),
            "biasqk": bias,
            "cs": np.ascontiguousarray(cs),
            "pit": pit,
        })
    return in_maps


_PROJ_W = None


def run_on_device(inputs, trace=False, trace_cores=None):
    """Shard, run on 8 NeuronCores, gather. Returns (output, BassKernelResults)."""
    global _PROJ_W
    from concourse import bass_utils

    x = np.asarray(inputs["x"], np.float32)
    cu = np.asarray(inputs["cu_seqlens"]).tolist()
    assert cu == [0, 512, 1024, 1536, 2048], (
        f"kernel compiled for 4x512 segments, got cu_seqlens={cu}")
    assert x.shape == (SEQ, 1, EMBED)

    _PROJ_W = np.asarray(inputs["proj_w"], np.float32)
    in_maps = _prep_inputs(x, inputs["rotary_pos_emb"],
                           inputs["qkv_w"], inputs["qkv_b"])

    if "nc" not in _CACHE:
        _CACHE["nc"] = _build_program()
    nc = _CACHE["nc"]

    kw = {}
    if trace:
        kw = dict(trace=True, trace_cores=trace_cores or [0])
    res = bass_utils.run_bass_kernel_spmd(nc, in_maps,
                                          core_ids=list(range(N_CORES)), **kw)

    proj_b = np.asarray(inputs["proj_b"], np.float32)
    out = np.empty((SEQ, EMBED), np.float32)
    for sg in range(2):
        acc = res.results[HPC * sg + 0]["outT"].astype(np.float32).copy()
        for hg in range(1, HPC):
            acc += res.results[HPC * sg + hg]["outT"].astype(np.float32)
        out[TOK * sg:TOK * (sg + 1)] = acc.T
    out += proj_b
    return out.reshape(SEQ, 1, EMBED), res


def kernel(**inputs):
    out, _ = run_on_device(inputs, trace=False)
    return out
